# revision 14
# baseline (speedup 1.0000x reference)
"""Trainium2 Bass kernel for nn_DeepModel3 (dense MLP, 47 layers).

Strategy: pure data parallel over 8 NeuronCores (batch 131072 -> 16384/core).
Activations are kept feature-major ([features, batch_cols]) on chip so every
layer is `h_next = W @ h` with the contraction on the partition dim. Narrow
layers (64/32/16 features) are packed: 2/4/8 independent batch streams are
stacked on the 128 partitions with block-diagonal weights, keeping the PE
array's K dim full. All matmuls run as float32r (full-rate fp32 mode).

Host-side prep (not on device): threshold w_custom, transpose x shards to
feature-major, pre-pack transposed / block-diagonal weights and bias columns.
"""

import sys
import types

import numpy as np

import concourse.bass as bass
import concourse.bacc as bacc
import concourse.mybir as mybir
from concourse import tile
from concourse.bass_utils import run_bass_kernel_spmd

N_CORES = 8
B = 131072
D = 256
BC = B // N_CORES          # per-core batch
THRESH = 0.01
F32 = mybir.dt.float32
F32R = mybir.dt.float32r
AF = mybir.ActivationFunctionType
ALU = mybir.AluOpType

SBB = 4096                 # superblock batch columns (8 chunks of 512)

# ---------------------------------------------------------------------------
# optional: make NTFF profiling available under this axon container (the
# shipped antenv stub lacks axon_hooks; run_bass_kernel_spmd(trace=True)
# imports it). Purely enables profiling; harmless if anything is missing.
def _install_ntff_shim():
    try:
        if "antenv.axon_hooks" not in sys.modules:
            import antenv  # noqa: F401
            mod = types.ModuleType("antenv.axon_hooks")
            mod._hook = None
            def set_axon_ntff_profile_hook(h):
                mod._hook = h
            def get_axon_ntff_profile_hook():
                return mod._hook
            mod.set_axon_ntff_profile_hook = set_axon_ntff_profile_hook
            mod.get_axon_ntff_profile_hook = get_axon_ntff_profile_hook
            sys.modules["antenv.axon_hooks"] = mod
            antenv.axon_hooks = mod
        m = sys.modules["antenv.axon_hooks"]
        if getattr(m, "_hook", None) is None:
            from trn_agent_boot.trn_boot import _ntff_profile_via_ctypes
            h = _ntff_profile_via_ctypes("/opt/axon/libaxon_pjrt.so")
            if h is not None:
                m.set_axon_ntff_profile_hook(h)
    except Exception:
        pass


_install_ntff_shim()


# ---------------------------------------------------------------------------
# host-side weight packing

def _bd(wt, copies):
    """Block-diagonal stack of `copies` copies of wt [k, m] -> [k*copies, m*copies]."""
    k, m = wt.shape
    out = np.zeros((k * copies, m * copies), np.float32)
    for i in range(copies):
        out[i * k:(i + 1) * k, i * m:(i + 1) * m] = wt
    return out


def pack_inputs(inputs):
    """Build the packed per-core weight/bias arrays (replicated on all cores)."""
    f = lambda a: np.asarray(a, np.float32)
    w_custom = f(inputs["w_custom"])
    w_custom = np.where(np.abs(w_custom) >= THRESH, w_custom, 0.0).astype(np.float32)
    big_ws = [w_custom] + [f(inputs["w_in"][i]) for i in range(3)]
    big_bs = [f(inputs["b_custom"])] + [f(inputs["b_in"][i]) for i in range(3)]

    # wbig [128, 4*4*128]: layer li, out-half m, k-chunk k at col (li*4+m*2+k)*128
    wbig = np.zeros((128, 4 * 512), np.float32)
    for li in range(4):
        wt = big_ws[li].T          # [Din, Dout] = lhsT
        for m in range(2):
            for k in range(2):
                col = li * 512 + m * 256 + k * 128
                wbig[:, col:col + 128] = wt[k * 128:(k + 1) * 128, m * 128:(m + 1) * 128]

    w4 = np.zeros((128, 128), np.float32)
    wt4 = f(inputs["w4"]).T        # [256, 64]
    for k in range(2):
        w4[:, k * 64:(k + 1) * 64] = wt4[k * 128:(k + 1) * 128, :]

    w64 = np.zeros((128, 21 * 128), np.float32)
    for l in range(21):
        w64[:, l * 128:(l + 1) * 128] = _bd(f(inputs["w64"][l]).T, 2)

    w26 = _bd(f(inputs["w26"]).T, 2)            # [128, 64]
    w32 = np.zeros((128, 9 * 128), np.float32)
    for l in range(9):
        w32[:, l * 128:(l + 1) * 128] = _bd(f(inputs["w32"][l]).T, 4)
    w36 = _bd(f(inputs["w36"]).T, 4)            # [128, 64]
    w16 = np.zeros((128, 10 * 128), np.float32)
    for l in range(10):
        w16[:, l * 128:(l + 1) * 128] = _bd(f(inputs["w16"][l]).T, 8)
    w47 = _bd(f(inputs["w47"]).T, 8)            # [128, 8]

    # bias columns [128, 52]
    bias = np.zeros((128, 52), np.float32)
    for li in range(4):
        for m in range(2):
            bias[:, li * 2 + m] = big_bs[li][m * 128:(m + 1) * 128]
    bias[:, 8] = np.tile(f(inputs["b4"]), 2)
    for l in range(21):
        bias[:, 9 + l] = np.tile(f(inputs["b64"][l]), 2)
    bias[:, 30] = np.tile(f(inputs["b26"]), 4)
    for l in range(9):
        bias[:, 31 + l] = np.tile(f(inputs["b32"][l]), 4)
    bias[:, 40] = np.tile(f(inputs["b36"]), 8)
    for l in range(10):
        bias[:, 41 + l] = np.tile(f(inputs["b16"][l]), 8)
    bias[0:8, 51] = np.tile(f(inputs["b47"]), 8)

    return {
        "wbig": wbig, "w4": w4, "w64": w64, "w26": w26, "w32": w32,
        "w36": w36, "w16": w16, "w47": w47, "bias": bias,
    }


BIAS_COL = {
    "big": lambda li, m: li * 2 + m,
    "fc4": 8,
    "b64": lambda l: 9 + l,
    "fc26": 30,
    "b32": lambda l: 31 + l,
    "fc36": 40,
    "b16": lambda l: 41 + l,
    "fc47": 51,
}


# ---------------------------------------------------------------------------
# kernel builder

def build(bc=BC):
    nc = bacc.Bacc(None, target_bir_lowering=False)
    xt = nc.declare_dram_parameter("xt", [D, bc], F32R, isOutput=False)
    wbig_d = nc.declare_dram_parameter("wbig", [128, 2048], F32R, isOutput=False)
    w4_d = nc.declare_dram_parameter("w4", [128, 128], F32R, isOutput=False)
    w64_d = nc.declare_dram_parameter("w64", [128, 21 * 128], F32R, isOutput=False)
    w26_d = nc.declare_dram_parameter("w26", [128, 64], F32R, isOutput=False)
    w32_d = nc.declare_dram_parameter("w32", [128, 9 * 128], F32R, isOutput=False)
    w36_d = nc.declare_dram_parameter("w36", [128, 64], F32R, isOutput=False)
    w16_d = nc.declare_dram_parameter("w16", [128, 10 * 128], F32R, isOutput=False)
    w47_d = nc.declare_dram_parameter("w47", [128, 8], F32R, isOutput=False)
    bias_d = nc.declare_dram_parameter("bias", [128, 52], F32, isOutput=False)
    out_d = nc.declare_dram_parameter("out", [bc], F32, isOutput=True)

    n_sb = bc // SBB

    # evict engine balancer (ns estimates; ACT @1.2GHz +222cyc, DVE @0.96 +120)
    bal = {"act": 0.0, "dve": 0.0}

    with tile.TileContext(nc) as tc:
        with (
            tc.tile_pool(name="wpool", bufs=1) as wpool,
            tc.tile_pool(name="xpool", bufs=2) as xpool,
            tc.tile_pool(name="hpool", bufs=3) as hpool,
            tc.tile_pool(name="pairpool", bufs=6) as pairpool,
            tc.tile_pool(name="quadpool", bufs=3) as quadpool,
            tc.tile_pool(name="octpool", bufs=3) as octpool,
            tc.tile_pool(name="outpool", bufs=2) as outpool,
            tc.tile_pool(name="psA", bufs=1, space="PSUM") as psA,
            tc.tile_pool(name="ps4", bufs=1, space="PSUM") as ps4,
            tc.tile_pool(name="psC", bufs=2, space="PSUM") as psC,
        ):
            # --- load weights & biases (resident) ---
            def wload(dram, shape, dt=F32R):
                t = wpool.tile(shape, dt, tag=dram.name)
                nc.sync.dma_start(out=t[:], in_=dram[:])
                return t

            wbig = wload(wbig_d, [128, 2048])
            w4 = wload(w4_d, [128, 128])
            w64 = wload(w64_d, [128, 21 * 128])
            w26 = wload(w26_d, [128, 64])
            w32 = wload(w32_d, [128, 9 * 128])
            w36 = wload(w36_d, [128, 64])
            w16 = wload(w16_d, [128, 10 * 128])
            w47 = wload(w47_d, [128, 8])
            bias_t = wload(bias_d, [128, 52], dt=F32)



            def bias_ap(col, rows=128, brow=0):
                return bias_t[brow:brow + rows, col:col + 1]

            def evict(ps_ap, out_ap, bcol, relu=True, rows=128, brow=0, force=None):
                fd = ps_ap.free_size()
                b = bias_ap(bcol, rows, brow)
                cost_a = (fd + 222) / 1.2
                cost_d = (fd + 120) / 0.96
                use_act = bal["act"] + cost_a <= bal["dve"] + cost_d
                if force is not None:
                    use_act = force == "act"
                if use_act:
                    bal["act"] += cost_a
                    if relu:
                        nc.scalar.activation(out_ap, ps_ap, AF.Relu, bias=b)
                    else:
                        nc.scalar.activation(out_ap, ps_ap, AF.Identity, bias=b)
                else:
                    bal["dve"] += cost_d
                    if relu:
                        nc.vector.tensor_scalar(out_ap, ps_ap, b, 0.0, ALU.add, ALU.max)
                    else:
                        nc.vector.tensor_scalar(out_ap, ps_ap, b, None, ALU.add)

            def mm(ps_ap, lhsT, rhs, start=True, stop=True):
                nc.tensor.matmul(ps_ap, lhsT, rhs, start=start, stop=stop)

            for sb in range(n_sb):
                base = sb * SBB
                P = [None, None]
                for p in range(2):
                    # ---- stage A: two chunk-halves (2048 batch cols) ----
                    hs = [[None, None], [None, None]]   # [half][kchunk]
                    for half in range(2):
                        c0 = base + (p * 2 + half) * 1024
                        for k in range(2):
                            xtile = xpool.tile([128, 1024], F32R, tag=f"x{half}{k}")
                            nc.sync.dma_start(
                                out=xtile[:], in_=xt[k * 128:(k + 1) * 128, c0:c0 + 1024])
                            hs[half][k] = xtile
                    for li in range(4):
                        for half in range(2):
                            nh = [None, None]
                            for m in range(2):
                                ps = psA.tile([128, 1024], F32, tag="psA")
                                for s in range(2):
                                    for k in range(2):
                                        col = li * 512 + m * 256 + k * 128
                                        mm(ps[:, s * 512:(s + 1) * 512],
                                           wbig[:, col:col + 128],
                                           hs[half][k][:, s * 512:(s + 1) * 512],
                                           start=(k == 0), stop=(k == 1))
                                ht = hpool.tile([128, 1024], F32R, tag=f"h{half}{m}")
                                evict(ps[:], ht[:], BIAS_COL["big"](li, m))
                                nh[m] = ht
                            hs[half] = nh
                    # ---- fc4: each half -> own [64,1024] psum, shifted evict ----
                    pt = pairpool.tile([128, 1024], F32R, tag="pair")
                    for half in range(2):
                        psp = ps4.tile([64, 1024], F32, tag="ps4")
                        for s in range(2):
                            for k in range(2):
                                mm(psp[:, s * 512:(s + 1) * 512],
                                   w4[:, k * 64:(k + 1) * 64],
                                   hs[half][k][:, s * 512:(s + 1) * 512],
                                   start=(k == 0), stop=(k == 1))
                        evict(psp[:], pt[64 * half:64 * (half + 1), :],
                              BIAS_COL["fc4"], rows=64, brow=64 * half)
                    P[p] = pt
                    # ---- fc5..fc25: 21 x 64-wide, 2 streams stacked ----
                    for l in range(21):
                        ps = psC.tile([128, 1024], F32, tag="psC")
                        for s in range(2):
                            mm(ps[:, s * 512:(s + 1) * 512],
                               w64[:, l * 128:(l + 1) * 128],
                               P[p][:, s * 512:(s + 1) * 512])
                        pt = pairpool.tile([128, 1024], F32R, tag="pair")
                        evict(ps[:], pt[:], BIAS_COL["b64"](l))
                        P[p] = pt
                # ---- fc26: per pair [64,1024] psum, shifted evict into quad ----
                Q = quadpool.tile([128, 1024], F32R, tag="quad")
                for p in range(2):
                    psq = psC.tile([64, 1024], F32, tag="psC")
                    for s in range(2):
                        mm(psq[:, s * 512:(s + 1) * 512],
                           w26[:, 0:64], P[p][:, s * 512:(s + 1) * 512])
                    evict(psq[:], Q[64 * p:64 * (p + 1), :],
                          BIAS_COL["fc26"], rows=64, brow=64 * p)
                # ---- fc27..fc35: 9 x 32-wide, 4 streams ----
                for l in range(9):
                    ps = psC.tile([128, 1024], F32, tag="psC")
                    for s in range(2):
                        mm(ps[:, s * 512:(s + 1) * 512],
                           w32[:, l * 128:(l + 1) * 128],
                           Q[:, s * 512:(s + 1) * 512])
                    Qn = quadpool.tile([128, 1024], F32R, tag="quad")
                    evict(ps[:], Qn[:], BIAS_COL["b32"](l))
                    Q = Qn
                # ---- fc36: per col-half [64,512] psum, shifted evict into oct ----
                O = octpool.tile([128, 512], F32R, tag="oct")
                for a in range(2):
                    pso = psC.tile([64, 1024], F32, tag="psC")
                    mm(pso[:, 0:512], w36[:, 0:64], Q[:, a * 512:(a + 1) * 512])
                    evict(pso[:, 0:512], O[64 * a:64 * (a + 1), :],
                          BIAS_COL["fc36"], rows=64, brow=64 * a)
                # ---- fc37..fc46: 10 x 16-wide, 8 streams ----
                for l in range(10):
                    ps = psC.tile([128, 1024], F32, tag="psC")
                    mm(ps[:, 0:512], w16[:, l * 128:(l + 1) * 128], O[:])
                    On = octpool.tile([128, 512], F32R, tag="oct")
                    evict(ps[:, 0:512], On[:], BIAS_COL["b16"](l))
                    O = On
                # ---- fc47 (no relu): [8, 512] ----
                ps = psC.tile([128, 1024], F32, tag="psC")
                mm(ps[0:8, 0:512], w47[:, 0:8], O[:])
                out_t = outpool.tile([128, 512], F32, tag="outt")
                evict(ps[0:8, 0:512], out_t[0:8, :],
                      BIAS_COL["fc47"], relu=False, rows=8, force="dve")
                # out flat = sb*4096 + b*1024 + a*512 + c ; out_t row = a*4 + b
                sbv = out_d[sb * SBB:(sb + 1) * SBB].rearrange(
                    "(b x) -> b x", b=4, x=1024)
                nc.sync.dma_start(out=sbv[:, 0:512], in_=out_t[0:4, :])
                nc.sync.dma_start(out=sbv[:, 512:1024], in_=out_t[4:8, :])

    nc.compile()
    return nc


_BUILT = {}


def get_nc(bc=BC):
    if bc not in _BUILT:
        _BUILT[bc] = build(bc)
    return _BUILT[bc]


# ---------------------------------------------------------------------------

LAST_RESULTS = None


def kernel(**inputs):
    """Full-input entry: shards x across 8 cores, runs the Bass kernel, gathers."""
    global LAST_RESULTS
    x = np.asarray(inputs["x"], np.float32)
    packed = pack_inputs(inputs)
    nc = get_nc(BC)
    in_maps = []
    for c in range(N_CORES):
        shard = np.ascontiguousarray(x[c * BC:(c + 1) * BC].T)   # [256, BC]
        m = {"xt": shard}
        m.update(packed)
        in_maps.append(m)
    res = run_bass_kernel_spmd(nc, in_maps, core_ids=list(range(N_CORES)))
    LAST_RESULTS = res
    out = np.concatenate([res.results[c]["out"] for c in range(N_CORES)])
    return out.reshape(B, 1).astype(np.float32)
